# revision 1
# baseline (speedup 1.0000x reference)
"""GAT layer (message passing) on 8 Trainium2 NeuronCores via Bass/Tile.

Strategy (src-sharded, dst-sectioned, K-padded node-aligned slots):
  - 8 cores, each owns nodes [c*NPC, (c+1)*NPC) (src sharding -> segment
    sums stay core-local).
  - Host passes x^T ROTATED per core so the core's own shard maps to table
    rows [0, NPC): all program constants become core-independent; per-core
    differences live only in index input data.
  - Phase 1 (per core, replicated work): compute per-node row
    G[n] = [h(64xbf16) | t(f32) | s(f32) | pad] via TensorE from xT chunks,
    where h = x@W, s = h@a1, t = h@a2.  G lives in DRAM, split in 4 sections
    of <=25088 rows (+ a special page per section with a TRASH row) so that
    int16 dma_gather indices stay in range.
  - Phase 2: per dst-section, per batch: dma_gather slot rows (256B) by dst,
    dma_gather s rows by node id, compute w = exp(leaky_relu(s+t)) and
    prod = w*h on ACT/DVE, reduce over each node's K slots (DVE), pack
    [h'(64) | e_sum | pad] rows and dma_scatter_add them into a DRAM
    accumulator indexed by local node id (unique per call -> no collisions;
    calls serialized by buffer reuse -> no cross-call races).
  - Final: load accumulator, out = h' * (1/e_sum), write [NPC_PAD, 64] f32.
"""

import numpy as np

N = 100000
E = 1600000
IN_C = 128
OUT_C = 64
ALPHA = 0.2
NCORES = 8
EL = 128                       # bf16 elements per table row (256B)
GB = 4                         # phase-1 chunks (of 128 nodes) per group
MAXC = 64                      # max slot-columns per batch (nidx<=8192)
MAXM = 64                      # max node-columns per batch
K_BUCKETS = (1, 2, 3, 4, 5, 6, 8, 10, 12, 14, 16, 20, 24, 28, 32,
             40, 48, 64, 96, 128)


def _configure(n=100000, e=1600000, secr_cap=25088):
    """Set the graph-size-derived constants (module globals)."""
    global N, E, NPC, NPC_PAD, ACC_TRASH, ACC_ROWS, SECR, NSEC
    global SECSTRIDE, XP, TROWS, XTILE
    N, E = n, e
    NPC = N // NCORES
    NPC_PAD = ((NPC + 127) // 128) * 128
    ACC_TRASH = NPC_PAD
    ACC_ROWS = NPC_PAD + 128
    NSEC = max(1, -(-N // secr_cap))
    per_sec = -(-N // NSEC)
    SECR = ((per_sec + 511) // 512) * 512   # multiple of 512 for phase-1 GB
    SECSTRIDE = SECR + 128
    XP = NSEC * SECR
    TROWS = NSEC * SECSTRIDE
    XTILE = min(4096, XP) if XP % 4096 == 0 or XP < 4096 else 4096
    assert XP % (128 * GB) == 0
    assert NPC_PAD < SECR + 128, "s-gather idx must stay in section 0 range"


_configure()


def _wrap16(vals):
    """[128, cols] rank layout (rank i -> (p=i%128, col=i//128)) ->
    dma_gather idx tensor [128, (128*cols)/16] int16 (wrapped, replicated)."""
    L = vals.T.reshape(-1)  # rank order
    n = L.shape[0]
    w = L.reshape(n // 16, 16).T  # [16, n/16]
    return np.tile(w, (8, 1)).astype(np.int16)


def _bucket(d):
    for k in K_BUCKETS:
        if d <= k:
            return k
    raise ValueError(f"degree {d} exceeds max bucket")


def _prep(edge_index):
    """Pure-integer host prep: per-core rotated sections, classes, batches.

    Returns (meta, per_core) where meta is core-independent (defines the
    program) and per_core holds the index input tensors."""
    src = np.asarray(edge_index[0], dtype=np.int64)
    dst = np.asarray(edge_index[1], dtype=np.int64)
    loops = np.arange(N, dtype=np.int64)
    src = np.concatenate([src, loops])
    dst = np.concatenate([dst, loops])

    core_of = src // NPC
    kb = np.asarray(K_BUCKETS)

    # ---- pass 1: per-core degree tables in rotated-section space ----
    per_core_raw = []
    # counts_psk[c][sec][ki][p] = #nodes of partition p in class ki
    counts_psk = np.zeros((NCORES, NSEC, len(kb), 128), np.int64)
    for c in range(NCORES):
        m = core_of == c
        sl = (src[m] - c * NPC).astype(np.int64)
        rot = (dst[m] - c * NPC) % N  # rotated dst position
        sec = rot // SECR
        row = (rot % SECR).astype(np.int32)
        key = sl * NSEC + sec
        order = np.argsort(key, kind="stable")
        rows_sorted = row[order]
        cnt = np.bincount(key, minlength=NPC * NSEC).reshape(NPC, NSEC)
        offs = np.zeros(NPC * NSEC + 1, np.int64)
        np.cumsum(cnt.ravel(), out=offs[1:])
        # class per (node, sec); -1 when no edges in that section
        cls = np.full((NPC, NSEC), -1, np.int64)
        nz = cnt > 0
        cls[nz] = np.searchsorted(kb, cnt[nz])
        for s_ in range(NSEC):
            for ki in range(len(kb)):
                n_in_class = int((cls[:, s_] == ki).sum())
                # nodes are distributed round-robin across partitions, so
                # per-partition count is at most ceil(n/128)
                counts_psk[c, s_, ki] = -(-n_in_class // 128)
        per_core_raw.append((cnt, cls, rows_sorted, offs))

    # ---- shared metadata: padded class sizes + batch structure ----
    m_sk = counts_psk.max(axis=(0, 3))  # [NSEC, nK]
    total_slots = int((m_sk * kb[None, :]).sum() * 128)
    total_edges = E + N
    # batches: list of dicts(sec, runs=[(ki, m_run)], cols, m)
    batches = []
    for s_ in range(NSEC):
        cur = {"sec": s_, "runs": [], "cols": 0, "m": 0}
        for ki in reversed(range(len(kb))):  # big K first
            K = int(kb[ki])
            m_left = int(m_sk[s_, ki])
            while m_left > 0:
                mfit_cols = (MAXC - cur["cols"]) // K
                mfit = min(m_left, mfit_cols, MAXM - cur["m"])
                if mfit <= 0:
                    if cur["runs"]:
                        batches.append(cur)
                    cur = {"sec": s_, "runs": [], "cols": 0, "m": 0}
                    continue
                cur["runs"].append((ki, mfit))
                cur["cols"] += mfit * K
                cur["m"] += mfit
                m_left -= mfit
        if cur["runs"]:
            batches.append(cur)
    meta = {"m_sk": m_sk, "batches": batches,
            "gcols": sum(b["cols"] for b in batches),
            "mcols": sum(b["m"] for b in batches),
            "inflation": total_slots / total_edges * NCORES}

    # ---- pass 2: fill per-core index tensors ----
    per_core = []
    for c in range(NCORES):
        cnt, cls, rows_sorted, offs = per_core_raw[c]
        p_of = np.arange(NPC) % 128
        # per (sec, ki): nodes grouped by partition, padded to m_sk
        gidx_blocks = []
        sidx_blocks = []
        nidx_blocks = []
        for b in batches:
            s_ = b["sec"]
            gmat = np.full((128, b["cols"]), SECR, np.int64)  # TRASH row
            smat = np.full((128, b["m"]), ACC_TRASH, np.int64)
            nmat = np.zeros((128, b["m"]), np.int64)  # s-gather idx
            b["_fill"] = (gmat, smat, nmat)
        run_index = []  # [(batch, col_off, m_off, ki, m_run)]
        for b in batches:
            co, mo = 0, 0
            for ki, m_run in b["runs"]:
                run_index.append((b, co, mo, ki, m_run))
                co += m_run * int(kb[ki])
                mo += m_run
        # group nodes per (sec, ki) with round-robin partition assignment:
        # node rank r in the class -> partition r%128, column r//128
        for s_ in range(NSEC):
            for ki in range(len(kb)):
                sel = np.where(cls[:, s_] == ki)[0]
                ncl = -(-len(sel) // 128) if len(sel) else 0
                nodes_by_p = [sel[p::128] for p in range(128)]
                K = int(kb[ki])
                consumed = np.zeros(128, np.int64)
                for b, co, mo, ki2, m_run in run_index:
                    if b["sec"] != s_ or ki2 != ki:
                        continue
                    gmat, smat, nmat = b["_fill"]
                    for p in range(128):
                        av = nodes_by_p[p][consumed[p]:consumed[p] + m_run]
                        nn = len(av)
                        if nn == 0:
                            continue
                        # node columns mo..mo+nn-1 on partition p
                        smat[p, mo:mo + nn] = av
                        nmat[p, mo:mo + nn] = av
                        # gather slots
                        d = cnt[av, s_]
                        base = offs[av * NSEC + s_]
                        idxmat = base[:, None] + np.arange(K)[None, :]
                        valid = np.arange(K)[None, :] < d[:, None]
                        vals = np.where(
                            valid,
                            rows_sorted[np.minimum(idxmat,
                                                   len(rows_sorted) - 1)],
                            SECR)
                        gmat[p, co + 0:co + nn * K] = vals.reshape(-1)
                    consumed += m_run
        # wrap all batches
        sorder_blocks = []
        for b in batches:
            gmat, smat, nmat = b["_fill"]
            gidx_blocks.append(_wrap16(gmat))
            sidx_blocks.append(_wrap16(smat))
            sorder_blocks.append(smat)
        per_core.append({
            "gidx": np.concatenate(gidx_blocks, axis=1),
            "sidx": np.concatenate(sidx_blocks, axis=1),
            "sorder": np.concatenate(sorder_blocks, axis=1),
        })
        for b in batches:
            del b["_fill"]
    return meta, per_core


def _build_program(meta, repeat=1, rep_mode="both", ablate=()):
    import concourse.bacc as bacc
    import concourse.tile as tile
    from concourse import mybir
    from concourse.masks import make_identity

    f32 = mybir.dt.float32
    bf16 = mybir.dt.bfloat16
    i16 = mybir.dt.int16
    AF = mybir.ActivationFunctionType
    OP = mybir.AluOpType
    AX = mybir.AxisListType
    kb = K_BUCKETS

    batches = meta["batches"]
    gcols_total = meta["gcols"]
    mcols_total = meta["mcols"]

    nc = bacc.Bacc(None, target_bir_lowering=False)
    xT = nc.dram_tensor("xT", [128, XP], f32, kind="ExternalInput")
    Wd = nc.dram_tensor("W", [IN_C, OUT_C], f32, kind="ExternalInput")
    aT = nc.dram_tensor("aT", [OUT_C, 2], f32, kind="ExternalInput")
    gidx = nc.dram_tensor("gidx", [128, gcols_total * 8], i16,
                          kind="ExternalInput")
    sidx = nc.dram_tensor("sidx", [128, mcols_total * 8], i16,
                          kind="ExternalInput")
    mcols_pad = -(-mcols_total // GB) * GB
    xS = nc.dram_tensor("xS", [128, mcols_pad * 128], f32,
                        kind="ExternalInput")
    acc = nc.dram_tensor("acc", [ACC_ROWS, EL], f32, kind="ExternalOutput")
    out = nc.dram_tensor("out", [NPC_PAD, OUT_C], f32, kind="ExternalOutput")

    with tile.TileContext(nc) as tc:
        with (
            tc.tile_pool(name="dram", bufs=1, space="DRAM") as dpool,
            tc.tile_pool(name="setup", bufs=1) as setup,
            tc.tile_pool(name="xin", bufs=2) as xin,
            tc.tile_pool(name="ps", bufs=2, space="PSUM") as psp,
            tc.tile_pool(name="gout", bufs=2) as gop,
            tc.tile_pool(name="ph2", bufs=2) as ph2,
            tc.tile_pool(name="hgp", bufs=2) as hgp,
            tc.tile_pool(name="ph2b", bufs=1) as ph2b,
            tc.tile_pool(name="fin", bufs=1) as fin,
        ):
            G = dpool.tile([TROWS, EL], bf16)

            # ---------- setup: W, Wa1, Wa2 ----------
            ident = setup.tile([128, 128], f32)
            make_identity(nc, ident[:])
            Wt = setup.tile([128, OUT_C], f32)
            nc.sync.dma_start(Wt[:], Wd[:])
            aTt = setup.tile([OUT_C, 2], f32)
            nc.sync.dma_start(aTt[:], aT[:])
            WT_ps = psp.tile([OUT_C, 128], f32, tag="wt")
            nc.tensor.transpose(out=WT_ps[:], in_=Wt[:], identity=ident[:])
            WT = setup.tile([OUT_C, 128], f32)
            nc.vector.tensor_copy(WT[:], WT_ps[:])
            Wa_ps = psp.tile([128, 2], f32, tag="wa")
            nc.tensor.matmul(Wa_ps[:], WT[:], aTt[:], start=True, stop=True)
            # rhs = [W | Wa2] (t rides in G); Wa1 separate for phase 1b (s)
            rhs = setup.tile([128, OUT_C + 1], f32)
            nc.vector.tensor_copy(rhs[:, 0:OUT_C], Wt[:])
            nc.vector.tensor_copy(rhs[:, OUT_C:OUT_C + 1], Wa_ps[:, 1:2])
            wa1 = setup.tile([128, 1], f32)
            nc.vector.tensor_copy(wa1[:], Wa_ps[:, 0:1])
            s_res = setup.tile([128, mcols_pad], f32)

            # special TRASH rows: h=0, t=-1e30, s=0
            sp = setup.tile([1, EL], bf16)
            nc.vector.memset(sp[:], 0)
            spf = sp[:].bitcast(f32)  # [1, 64]
            nc.vector.memset(spf[0:1, 32:33], -1.0e30)
            for s_ in range(NSEC):
                nc.sync.dma_start(G[s_ * SECSTRIDE + SECR:
                                    s_ * SECSTRIDE + SECR + 1, :], sp[:])

            for _rep in range(repeat):
                do_p1 = rep_mode in ("both", "p1") or _rep == 0
                do_p2 = rep_mode in ("both", "p2") or _rep == 0
                # ---------- phase 1: G rows ----------
                n_groups = (XP // (128 * GB)) if do_p1 else 0
                groups_per_xtile = max(1, XTILE // (128 * GB))  # 8
                xt_t = None
                for g in range(n_groups):
                    if g % groups_per_xtile == 0:
                        xt_t = xin.tile([128, XTILE], f32, tag="xt")
                        x0 = g * 128 * GB
                        xlen = min(XTILE, XP - x0)
                        nc.sync.dma_start(xt_t[:, 0:xlen], xT[:, x0:x0 + xlen])
                    ps = psp.tile([128, GB, OUT_C + 1], f32, tag="hps")
                    for j in range(GB):
                        off = (g % groups_per_xtile) * 128 * GB + j * 128
                        nc.tensor.matmul(ps[:, j, :], xt_t[:, off:off + 128],
                                         rhs[:], start=True, stop=True)
                    gb_t = gop.tile([128, GB, EL], bf16, tag="gb")
                    nc.vector.tensor_copy(gb_t[:, :, 0:OUT_C], ps[:, :, 0:OUT_C])
                    gf = gb_t[:].bitcast(f32)  # [128, GB, 64]
                    # t (= h@a2) at f32 col 32
                    nc.vector.tensor_copy(gf[:, :, 32:33],
                                          ps[:, :, OUT_C:OUT_C + 1])
                    # write 512 rows: node = 512 g + j*128 + p
                    sec = (g * 512) // SECR
                    rowbase = (g * 512) % SECR + sec * SECSTRIDE
                    nc.sync.dma_start(
                        G[rowbase:rowbase + 512, :].rearrange(
                            "(j p) e -> p j e", p=128),
                        gb_t[:])

                # ---------- phase 1b: s in slot-node order ----------
                if do_p1:
                    n_sgroups = mcols_pad // GB
                    for g in range(n_sgroups):
                        xs_t = xin.tile([128, GB * 128], f32, tag="xs")
                        nc.sync.dma_start(
                            xs_t[:], xS[:, g * GB * 128:(g + 1) * GB * 128])
                        ps2 = psp.tile([128, GB, 1], f32, tag="sps")
                        for j in range(GB):
                            nc.tensor.matmul(
                                ps2[:, j, :], xs_t[:, j * 128:(j + 1) * 128],
                                wa1[:], start=True, stop=True)
                        nc.vector.tensor_copy(
                            s_res[:, g * GB:(g + 1) * GB], ps2[:, :, 0])

                # ---------- phase 2 ----------
                go = 0  # offset into gidx (wrapped columns)
                mo = 0
                for b in (batches if do_p2 else []):
                    cols, mb, sec = b["cols"], b["m"], b["sec"]
                    ncols16 = cols * 8   # = (128*cols)/16
                    mcols16 = mb * 8
                    git = ph2.tile([128, 512], i16, tag="git")
                    nc.sync.dma_start(git[:, 0:ncols16],
                                      gidx[:, go:go + ncols16])
                    sit = ph2.tile([128, 512], i16, tag="sit")
                    nc.sync.dma_start(sit[:, 0:mcols16],
                                      sidx[:, mo:mo + mcols16])
                    hg = hgp.tile([128, MAXC, EL], bf16, tag="hg")
                    hg_n = cols if "hgather" not in ablate else 1
                    nc.gpsimd.dma_gather(
                        out_ap=hg[:, 0:hg_n, :],
                        in_ap=G[sec * SECSTRIDE:(sec + 1) * SECSTRIDE, :],
                        idxs_ap=git[:, 0:hg_n * 8],
                        num_idxs=128 * hg_n, num_idxs_reg=128 * hg_n,
                        elem_size=EL, single_packet=False)

                    do_dve = "dve" not in ablate
                    hgf = hg[:].bitcast(f32)     # [128, MAXC, 64]
                    z = ph2.tile([128, MAXC], f32, tag="z")
                    co2, mo2 = 0, 0
                    for ki, m_run in (b["runs"] if do_dve else []):
                        K = int(kb[ki])
                        t4 = hgf[:, co2:co2 + m_run * K, 32:33].rearrange(
                            "p (m k) e -> p m k e", k=K)
                        s4 = s_res[:, mo // 8 + mo2:mo // 8 + mo2 + m_run][
                            :, :, None, None]
                        z4 = z[:, co2:co2 + m_run * K].rearrange(
                            "p (m k) -> p m k", k=K)[:, :, :, None]
                        nc.vector.tensor_tensor(
                            out=z4, in0=t4,
                            in1=s4.to_broadcast([128, m_run, K, 1]), op=OP.add)
                        co2 += m_run * K
                        mo2 += m_run
                    zm = ph2.tile([128, MAXC], f32, tag="zm")
                    zl = ph2.tile([128, MAXC], f32, tag="zl")
                    w = ph2.tile([128, MAXC], bf16, tag="w")
                    prod = ph2.tile([128, MAXC, OUT_C], bf16, tag="prod")
                    if do_dve:
                        nc.vector.tensor_scalar_mul(zm[:, 0:cols], z[:, 0:cols], ALPHA)
                        nc.vector.tensor_tensor(out=zl[:, 0:cols], in0=z[:, 0:cols],
                                                in1=zm[:, 0:cols], op=OP.max)
                        nc.scalar.activation(w[:, 0:cols], zl[:, 0:cols], AF.Exp)
                        nc.vector.tensor_tensor(
                            out=prod[:, 0:cols, :], in0=hg[:, 0:cols, 0:OUT_C],
                            in1=w[:, 0:cols, None].to_broadcast([128, cols, OUT_C]),
                            op=OP.mult)
                    partial = ph2b.tile([128, MAXM, EL], f32, tag="partial")
                    if not do_dve:
                        nc.vector.memset(partial[:, 0:mb, 0:1], 0)
                    co2, mo2 = 0, 0
                    for ki, m_run in (b["runs"] if do_dve else []):
                        K = int(kb[ki])
                        pv = prod[:, co2:co2 + m_run * K, :].rearrange(
                            "p (m k) e -> p m e k", k=K)
                        nc.vector.tensor_reduce(
                            out=partial[:, mo2:mo2 + m_run, 0:OUT_C], in_=pv,
                            axis=AX.X, op=OP.add)
                        wv = w[:, co2:co2 + m_run * K].rearrange(
                            "p (m k) -> p m k", k=K)
                        nc.vector.tensor_reduce(
                            out=partial[:, mo2:mo2 + m_run, OUT_C:OUT_C + 1],
                            in_=wv, axis=AX.X, op=OP.add)
                        co2 += m_run * K
                        mo2 += m_run
                    if "scatter" not in ablate:
                        nc.gpsimd.dma_scatter_add(
                            out_ap=acc[:], in_ap=partial[:, 0:mb, :],
                            idxs_ap=sit[:, 0:mcols16],
                            num_idxs=128 * mb, num_idxs_reg=128 * mb,
                            elem_size=EL, single_packet=False)
                    go += ncols16
                    mo += mcols16

            # ---------- final: out = h' / e_sum ----------
            nacc_cols = ACC_ROWS // 128  # 99
            at = fin.tile([128, nacc_cols, EL], f32)
            nc.sync.dma_start(
                at[:], acc[:].rearrange("(j p) e -> p j e", p=128))
            rec = fin.tile([128, nacc_cols], f32)
            nc.vector.reciprocal(rec[:], at[:, :, OUT_C:OUT_C + 1])
            ot = fin.tile([128, NPC_PAD // 128, OUT_C], f32)
            nc.vector.tensor_tensor(
                out=ot[:], in0=at[:, 0:NPC_PAD // 128, 0:OUT_C],
                in1=rec[:, 0:NPC_PAD // 128, None].to_broadcast(
                    [128, NPC_PAD // 128, OUT_C]),
                op=OP.mult)
            nc.sync.dma_start(
                out[:].rearrange("(j p) e -> p j e", p=128), ot[:])
    nc.compile()
    return nc


_CACHE = {}
_LAST = {}  # debug/timing introspection: nc + in_maps of last call


def kernel(x, W, a, edge_index):
    from concourse.bass_utils import run_bass_kernel_spmd

    x = np.asarray(x, np.float32)
    W = np.asarray(W, np.float32)
    a = np.asarray(a, np.float32)
    meta, per_core = _prep(edge_index)

    key = (N, E, tuple((b["sec"], tuple(b["runs"])) for b in meta["batches"]))
    if key not in _CACHE:
        _CACHE[key] = _build_program(meta)
    nc = _CACHE[key]

    xTf = np.ascontiguousarray(x.T)  # [128, N]
    aTv = np.ascontiguousarray(a.reshape(2, OUT_C).T)  # [64, 2]
    mcols_total = meta["mcols"]
    mcols_pad = -(-mcols_total // GB) * GB
    in_maps = []
    for c in range(NCORES):
        n0 = c * NPC
        xrot = np.concatenate(
            [xTf[:, n0:], xTf[:, :n0],
             np.zeros((128, XP - N), np.float32)], axis=1)
        ids = per_core[c]["sorder"].T.reshape(-1)  # q = j*128+p
        xs = np.zeros((mcols_pad * 128, IN_C), np.float32)
        valid = ids < NPC
        xs[:len(ids)][valid] = x[n0 + ids[valid]]
        in_maps.append({
            "xT": xrot, "W": W, "aT": aTv,
            "xS": np.ascontiguousarray(xs.T),
            "gidx": per_core[c]["gidx"],
            "sidx": per_core[c]["sidx"],
        })
    _LAST["nc"] = nc
    _LAST["in_maps"] = in_maps
    res = run_bass_kernel_spmd(nc, in_maps, core_ids=list(range(NCORES)))
    outs = [res.results[c]["out"][:NPC] for c in range(NCORES)]
    return np.concatenate(outs, axis=0)



# revision 21
# speedup vs baseline: 25.5802x; 25.5802x over previous
"""GAT layer (message passing) on 8 Trainium2 NeuronCores via Bass/Tile.

Strategy (src-sharded, dst-sectioned, K-padded node-aligned slots):
  - 8 cores, each owns nodes [c*NPC, (c+1)*NPC) (src sharding -> segment
    sums stay core-local).
  - Host passes x^T ROTATED per core so the core's own shard maps to table
    rows [0, NPC): all program constants become core-independent; per-core
    differences live only in index input data.
  - Phase 1 (per core, replicated work): compute per-node row
    G[n] = [h(64xbf16) | t(f32) | s(f32) | pad] via TensorE from xT chunks,
    where h = x@W, s = h@a1, t = h@a2.  G lives in DRAM, split in 4 sections
    of <=25088 rows (+ a special page per section with a TRASH row) so that
    int16 dma_gather indices stay in range.
  - Phase 2: per dst-section, per batch: dma_gather slot rows (256B) by dst,
    dma_gather s rows by node id, compute w = exp(leaky_relu(s+t)) and
    prod = w*h on ACT/DVE, reduce over each node's K slots (DVE), pack
    [h'(64) | e_sum | pad] rows and dma_scatter_add them into a DRAM
    accumulator indexed by local node id (unique per call -> no collisions;
    calls serialized by buffer reuse -> no cross-call races).
  - Final: load accumulator, out = h' * (1/e_sum), write [NPC_PAD, 64] f32.
"""

import numpy as np

N = 100000
E = 1600000
IN_C = 128
OUT_C = 64
ALPHA = 0.2
NCORES = 8
EL = 128                       # bf16 elements per table row (256B)
GB = 4                         # phase-1 chunks (of 128 nodes) per group
MAXC = 64                      # max slot-columns per batch (nidx<=8192)
MAXM = 64                      # max node-columns per batch
K_BUCKETS = (1, 2, 3, 4, 5, 6, 8, 10, 12, 14, 16, 20, 24, 28, 32,
             40, 48, 64, 96, 128)


def _configure(n=100000, e=1600000, secr_cap=25088):
    """Set the graph-size-derived constants (module globals)."""
    global N, E, NPC, NPC_PAD, ACC_TRASH, ACC_ROWS, SECR, NSEC
    global SECSTRIDE, XP, TROWS, XTILE
    N, E = n, e
    NPC = N // NCORES
    NPC_PAD = ((NPC + 127) // 128) * 128
    ACC_TRASH = NPC_PAD
    ACC_ROWS = NPC_PAD + 128
    NSEC = max(1, -(-N // secr_cap))
    per_sec = -(-N // NSEC)
    SECR = ((per_sec + 511) // 512) * 512   # multiple of 512 for phase-1 GB
    SECSTRIDE = SECR + 128
    XP = NSEC * SECR
    TROWS = NSEC * SECSTRIDE
    XTILE = min(2048, XP)
    assert XP % (128 * GB) == 0
    assert NPC_PAD < SECR + 128, "s-gather idx must stay in section 0 range"


_configure()


def _wrap16(vals):
    """[128, cols] rank layout (rank i -> (p=i%128, col=i//128)) ->
    dma_gather idx tensor [128, (128*cols)/16] int16 (wrapped, replicated)."""
    L = vals.T.reshape(-1)  # rank order
    n = L.shape[0]
    w = L.reshape(n // 16, 16).T  # [16, n/16]
    return np.tile(w, (8, 1)).astype(np.int16)


def _bucket(d):
    for k in K_BUCKETS:
        if d <= k:
            return k
    raise ValueError(f"degree {d} exceeds max bucket")


def _prep(edge_index):
    """Pure-integer host prep: per-core rotated sections, classes, batches.

    Returns (meta, per_core) where meta is core-independent (defines the
    program) and per_core holds the index input tensors."""
    src = np.asarray(edge_index[0], dtype=np.int64)
    dst = np.asarray(edge_index[1], dtype=np.int64)
    loops = np.arange(N, dtype=np.int64)
    src = np.concatenate([src, loops])
    dst = np.concatenate([dst, loops])

    core_of = src // NPC
    kb = np.asarray(K_BUCKETS)

    # ---- pass 1: per-core degree tables in rotated-section space ----
    per_core_raw = []
    # counts_psk[c][sec][ki][p] = #nodes of partition p in class ki
    counts_psk = np.zeros((NCORES, NSEC, len(kb), 128), np.int64)
    for c in range(NCORES):
        m = core_of == c
        sl = (src[m] - c * NPC).astype(np.int64)
        rot = (dst[m] - c * NPC) % N  # rotated dst position
        sec = rot // SECR
        row = (rot % SECR).astype(np.int32)
        key = sl * NSEC + sec
        order = np.argsort(key, kind="stable")
        rows_sorted = row[order]
        cnt = np.bincount(key, minlength=NPC * NSEC).reshape(NPC, NSEC)
        offs = np.zeros(NPC * NSEC + 1, np.int64)
        np.cumsum(cnt.ravel(), out=offs[1:])
        # class per (node, sec); -1 when no edges in that section
        cls = np.full((NPC, NSEC), -1, np.int64)
        nz = cnt > 0
        cls[nz] = np.searchsorted(kb, cnt[nz])
        for s_ in range(NSEC):
            for ki in range(len(kb)):
                n_in_class = int((cls[:, s_] == ki).sum())
                # nodes are distributed round-robin across partitions, so
                # per-partition count is at most ceil(n/128)
                counts_psk[c, s_, ki] = -(-n_in_class // 128)
        per_core_raw.append((cnt, cls, rows_sorted, offs))

    # ---- shared metadata: padded class sizes + batch structure ----
    m_sk = counts_psk.max(axis=(0, 3))  # [NSEC, nK]
    total_slots = int((m_sk * kb[None, :]).sum() * 128)
    total_edges = E + N
    # batches: list of dicts(sec, runs=[(ki, m_run)], cols, m)
    batches = []
    for s_ in range(NSEC):
        cur = {"sec": s_, "runs": [], "cols": 0, "m": 0}
        for ki in reversed(range(len(kb))):  # big K first
            K = int(kb[ki])
            m_left = int(m_sk[s_, ki])
            while m_left > 0:
                mfit_cols = (MAXC - cur["cols"]) // K
                mfit = min(m_left, mfit_cols, MAXM - cur["m"])
                if mfit <= 0:
                    if cur["runs"]:
                        batches.append(cur)
                    cur = {"sec": s_, "runs": [], "cols": 0, "m": 0}
                    continue
                cur["runs"].append((ki, mfit))
                cur["cols"] += mfit * K
                cur["m"] += mfit
                m_left -= mfit
        if cur["runs"]:
            batches.append(cur)
    meta = {"m_sk": m_sk, "batches": batches,
            "gcols": sum(b["cols"] for b in batches),
            "mcols": sum(b["m"] for b in batches),
            "inflation": total_slots / total_edges * NCORES}

    # ---- pass 2: fill per-core index tensors ----
    per_core = []
    for c in range(NCORES):
        cnt, cls, rows_sorted, offs = per_core_raw[c]
        p_of = np.arange(NPC) % 128
        # per (sec, ki): nodes grouped by partition, padded to m_sk
        gidx_blocks = []
        sidx_blocks = []
        nidx_blocks = []
        for b in batches:
            s_ = b["sec"]
            gmat = np.full((128, b["cols"]), SECR, np.int64)  # TRASH row
            smat = np.full((128, b["m"]), ACC_TRASH, np.int64)
            nmat = np.zeros((128, b["m"]), np.int64)  # s-gather idx
            b["_fill"] = (gmat, smat, nmat)
        run_index = []  # [(batch, col_off, m_off, ki, m_run)]
        for b in batches:
            co, mo = 0, 0
            for ki, m_run in b["runs"]:
                run_index.append((b, co, mo, ki, m_run))
                co += m_run * int(kb[ki])
                mo += m_run
        # group nodes per (sec, ki) with round-robin partition assignment:
        # node rank r in the class -> partition r%128, column r//128
        for s_ in range(NSEC):
            for ki in range(len(kb)):
                sel = np.where(cls[:, s_] == ki)[0]
                ncl = -(-len(sel) // 128) if len(sel) else 0
                nodes_by_p = [sel[p::128] for p in range(128)]
                K = int(kb[ki])
                consumed = np.zeros(128, np.int64)
                for b, co, mo, ki2, m_run in run_index:
                    if b["sec"] != s_ or ki2 != ki:
                        continue
                    gmat, smat, nmat = b["_fill"]
                    for p in range(128):
                        av = nodes_by_p[p][consumed[p]:consumed[p] + m_run]
                        nn = len(av)
                        if nn == 0:
                            continue
                        # node columns mo..mo+nn-1 on partition p
                        smat[p, mo:mo + nn] = av
                        nmat[p, mo:mo + nn] = av
                        # gather slots
                        d = cnt[av, s_]
                        base = offs[av * NSEC + s_]
                        idxmat = base[:, None] + np.arange(K)[None, :]
                        valid = np.arange(K)[None, :] < d[:, None]
                        vals = np.where(
                            valid,
                            rows_sorted[np.minimum(idxmat,
                                                   len(rows_sorted) - 1)],
                            SECR)
                        gmat[p, co + 0:co + nn * K] = vals.reshape(-1)
                    consumed += m_run
        # wrap all batches
        sorder_blocks = []
        for b in batches:
            gmat, smat, nmat = b["_fill"]
            gidx_blocks.append(_wrap16(gmat))
            sidx_blocks.append(_wrap16(smat))
            sorder_blocks.append(smat)
        per_core.append({
            "gidx": np.concatenate(gidx_blocks, axis=1),
            "sidx": np.concatenate(sidx_blocks, axis=1),
            "sorder": np.concatenate(sorder_blocks, axis=1),
        })
        for b in batches:
            del b["_fill"]
    return meta, per_core


def _build_program(meta, repeat=1, rep_mode="both", ablate=()):
    import concourse.bacc as bacc
    import concourse.tile as tile
    from concourse import mybir
    from concourse.masks import make_identity

    f32 = mybir.dt.float32
    bf16 = mybir.dt.bfloat16
    i16 = mybir.dt.int16
    AF = mybir.ActivationFunctionType
    OP = mybir.AluOpType
    AX = mybir.AxisListType
    kb = K_BUCKETS

    batches = meta["batches"]
    gcols_total = meta["gcols"]
    mcols_total = meta["mcols"]
    nbat = len(batches)
    # prefix offsets per batch (wrapped-idx cols and node cols)
    goffs = np.concatenate([[0], np.cumsum([b["cols"] * 8 for b in batches])])
    moffs = np.concatenate([[0], np.cumsum([b["m"] * 8 for b in batches])])

    nc = bacc.Bacc(None, target_bir_lowering=False)
    xT = nc.dram_tensor("xT", [128, XP], f32, kind="ExternalInput")
    Wd = nc.dram_tensor("W", [IN_C, OUT_C], f32, kind="ExternalInput")
    aT = nc.dram_tensor("aT", [OUT_C, 2], f32, kind="ExternalInput")
    gidx = nc.dram_tensor("gidx", [128, gcols_total * 8], i16,
                          kind="ExternalInput")
    sidx = nc.dram_tensor("sidx", [128, mcols_total * 8], i16,
                          kind="ExternalInput")
    mcols_pad = -(-mcols_total // GB) * GB
    xS = nc.dram_tensor("xS", [128, mcols_pad * 128], f32,
                        kind="ExternalInput")
    # bf16 accumulator rows: [h'(64) | e_sum | pad] = 128 bf16 = 256B.
    # Two accumulators (even/odd batches): concurrent in-flight scatter_adds
    # never touch the same tensor, and same-parity scatters are serialized
    # by partial-buffer reuse (bufs=2) -> no RMW races.
    acc0 = nc.dram_tensor("acc0", [ACC_ROWS, EL], bf16, kind="ExternalOutput")
    acc1 = nc.dram_tensor("acc1", [ACC_ROWS, EL], bf16, kind="ExternalOutput")
    out = nc.dram_tensor("out", [NPC_PAD, OUT_C], f32, kind="ExternalOutput")

    with tile.TileContext(nc) as tc:
        with (
            tc.tile_pool(name="dram", bufs=1, space="DRAM") as dpool,
            tc.tile_pool(name="setup", bufs=1) as setup,
            tc.tile_pool(name="xin", bufs=2) as xin,
            tc.tile_pool(name="ps", bufs=2, space="PSUM") as psp,
            tc.tile_pool(name="gout", bufs=2) as gop,
            tc.tile_pool(name="ph2", bufs=3) as ph2,
            tc.tile_pool(name="hgp", bufs=4) as hgp,
            tc.tile_pool(name="ph2b", bufs=2) as ph2b,
            tc.tile_pool(name="pfp", bufs=1) as pfp,
            tc.tile_pool(name="fin", bufs=1) as fin,
        ):
            Gs = []
            for s_ in range(NSEC):
                gsec = dpool.tile([SECSTRIDE, EL], bf16, tag=f"gsec{s_}",
                                  name=f"gsec{s_}")
                Gs.append(gsec)

            # ---------- setup: W, Wa1, Wa2 ----------
            ident = setup.tile([128, 128], f32)
            make_identity(nc, ident[:])
            Wt = setup.tile([128, OUT_C], f32)
            nc.sync.dma_start(Wt[:], Wd[:])
            aTt = setup.tile([OUT_C, 2], f32)
            nc.sync.dma_start(aTt[:], aT[:])
            WT_ps = psp.tile([OUT_C, 128], f32, tag="wt")
            nc.tensor.transpose(out=WT_ps[:], in_=Wt[:], identity=ident[:])
            WT = setup.tile([OUT_C, 128], f32)
            nc.vector.tensor_copy(WT[:], WT_ps[:])
            Wa_ps = psp.tile([128, 2], f32, tag="wa")
            nc.tensor.matmul(Wa_ps[:], WT[:], aTt[:], start=True, stop=True)
            # rhs = [W | Wa2] (t rides in G); Wa1 separate for phase 1b (s)
            rhs = setup.tile([128, OUT_C + 1], f32)
            nc.vector.tensor_copy(rhs[:, 0:OUT_C], Wt[:])
            nc.vector.tensor_copy(rhs[:, OUT_C:OUT_C + 1], Wa_ps[:, 1:2])
            wa1 = setup.tile([128, 1], f32)
            nc.vector.tensor_copy(wa1[:], Wa_ps[:, 0:1])
            # per-section s tiles so early batches don't wait on all of 1b
            sec_start, secs = {}, []
            for k, b in enumerate(batches):
                if b["sec"] not in sec_start:
                    sec_start[b["sec"]] = moffs[k] // 8
                    secs.append(b["sec"])
            sec_end = {
                s_: (sec_start[secs[i + 1]] if i + 1 < len(secs)
                     else mcols_total)
                for i, s_ in enumerate(secs)
            }
            s_tiles = {}
            for s_ in secs:
                stile = setup.tile(
                    [128, sec_end[s_] - sec_start[s_]], f32,
                    tag=f"sres{s_}", name=f"sres{s_}")
                s_tiles[s_] = stile

            # special TRASH rows: h=0, t=-1e30, s=0
            sp = setup.tile([1, EL], bf16)
            nc.vector.memset(sp[:], 0)
            spf = sp[:].bitcast(f32)  # [1, 64]
            nc.vector.memset(spf[0:1, 32:33], -1.0e30)
            for s_ in range(NSEC):
                nc.sync.dma_start(Gs[s_][SECR:SECR + 1, :], sp[:])

            # ---------- phase 1b first: s in slot-node order ----------
            n_sgroups = mcols_pad // GB
            for g in range(n_sgroups):
                xs_t = xin.tile([128, GB * 128], f32, tag="xs")
                nc.sync.dma_start(
                    xs_t[:], xS[:, g * GB * 128:(g + 1) * GB * 128])
                ps2 = psp.tile([128, GB, 1], f32, tag="sps")
                for j in range(GB):
                    nc.tensor.matmul(
                        ps2[:, j, :], xs_t[:, j * 128:(j + 1) * 128],
                        wa1[:], start=True, stop=True)
                # route each column to its section's s tile
                g0 = g * GB
                for s_ in secs:
                    lo = max(g0, sec_start[s_])
                    hi = min(g0 + GB, sec_end[s_])
                    if lo < hi:
                        nc.vector.tensor_copy(
                            s_tiles[s_][:, lo - sec_start[s_]:
                                        hi - sec_start[s_]],
                            ps2[:, lo - g0:hi - g0, 0])

            # ---------- phase 1: G rows ----------
            n_groups = XP // (128 * GB)
            groups_per_xtile = max(1, XTILE // (128 * GB))  # 8
            xt_t = None
            for g in range(n_groups):
                if g % groups_per_xtile == 0:
                    xt_t = xin.tile([128, XTILE], f32, tag="xt")
                    x0 = g * 128 * GB
                    xlen = min(XTILE, XP - x0)
                    nc.sync.dma_start(xt_t[:, 0:xlen], xT[:, x0:x0 + xlen])
                ps = psp.tile([128, GB, OUT_C + 1], f32, tag="hps")
                for j in range(GB):
                    off = (g % groups_per_xtile) * 128 * GB + j * 128
                    nc.tensor.matmul(ps[:, j, :], xt_t[:, off:off + 128],
                                     rhs[:], start=True, stop=True)
                gb_t = gop.tile([128, GB, EL], bf16, tag="gb")
                nc.vector.tensor_copy(gb_t[:, :, 0:OUT_C], ps[:, :, 0:OUT_C])
                gf = gb_t[:].bitcast(f32)  # [128, GB, 64]
                # t (= h@a2) at f32 col 32
                nc.vector.tensor_copy(gf[:, :, 32:33],
                                      ps[:, :, OUT_C:OUT_C + 1])
                # write 512 rows: node = 512 g + j*128 + p
                sec = (g * 512) // SECR
                rowbase = (g * 512) % SECR
                nc.sync.dma_start(
                    Gs[sec][rowbase:rowbase + 512, :].rearrange(
                        "(j p) e -> p j e", p=128),
                    gb_t[:])

            # ---------- phase 2 (software-pipelined, prep+trigger) ----------
            def emit_gather(k):
                b = batches[k]
                cols, sec = b["cols"], b["sec"]
                ncols16 = cols * 8
                git = ph2.tile([128, 512], i16, tag="git")
                nc.sync.dma_start(git[:, 0:ncols16],
                                  gidx[:, goffs[k]:goffs[k] + ncols16])
                hg = hgp.tile([128, MAXC, EL], bf16, tag="hg")
                nc.gpsimd.dma_gather(
                    out_ap=hg[:, 0:cols, :],
                    in_ap=Gs[sec][:],
                    idxs_ap=git[:, 0:cols * 8],
                    num_idxs=128 * cols, num_idxs_reg=128 * cols,
                    elem_size=EL, single_packet=False)
                return hg

            def emit_compute_scatter(k, hg):
                b = batches[k]
                cols, mb = b["cols"], b["m"]
                mcols16 = mb * 8
                mo8 = moffs[k] // 8
                sit = ph2.tile([128, 512], i16, tag="sit")
                nc.sync.dma_start(sit[:, 0:mcols16],
                                  sidx[:, moffs[k]:moffs[k] + mcols16])
                s_sec = s_tiles[b["sec"]]
                ml = mo8 - sec_start[b["sec"]]
                hgf = hg[:].bitcast(f32)     # [128, MAXC, 64]
                z = ph2.tile([128, MAXC], f32, tag="z")
                co2, mo2 = 0, 0
                for ki, m_run in b["runs"]:
                    K = int(kb[ki])
                    t4 = hgf[:, co2:co2 + m_run * K, 32:33].rearrange(
                        "p (m k) e -> p m k e", k=K)
                    s4 = s_sec[:, ml + mo2:ml + mo2 + m_run][
                        :, :, None, None]
                    z4 = z[:, co2:co2 + m_run * K].rearrange(
                        "p (m k) -> p m k", k=K)[:, :, :, None]
                    nc.vector.tensor_tensor(
                        out=z4, in0=t4,
                        in1=s4.to_broadcast([128, m_run, K, 1]), op=OP.add)
                    co2 += m_run * K
                    mo2 += m_run
                zm = ph2.tile([128, MAXC], f32, tag="zm")
                zl = ph2.tile([128, MAXC], f32, tag="zl")
                w = ph2.tile([128, MAXC], bf16, tag="w")
                prod = ph2.tile([128, MAXC, OUT_C], bf16, tag="prod")
                nc.vector.tensor_scalar_mul(zm[:, 0:cols], z[:, 0:cols],
                                            ALPHA)
                nc.vector.tensor_tensor(out=zl[:, 0:cols], in0=z[:, 0:cols],
                                        in1=zm[:, 0:cols], op=OP.max)
                nc.scalar.activation(w[:, 0:cols], zl[:, 0:cols], AF.Exp)
                nc.vector.tensor_tensor(
                    out=prod[:, 0:cols, :], in0=hg[:, 0:cols, 0:OUT_C],
                    in1=w[:, 0:cols, None].to_broadcast([128, cols, OUT_C]),
                    op=OP.mult)
                pf = pfp.tile([128, MAXM, OUT_C + 1], f32, tag="pf")
                co2, mo2 = 0, 0
                for ki, m_run in b["runs"]:
                    K = int(kb[ki])
                    pv = prod[:, co2:co2 + m_run * K, :].rearrange(
                        "p (m k) e -> p m e k", k=K)
                    nc.vector.tensor_reduce(
                        out=pf[:, mo2:mo2 + m_run, 0:OUT_C], in_=pv,
                        axis=AX.X, op=OP.add)
                    wv = w[:, co2:co2 + m_run * K].rearrange(
                        "p (m k) -> p m k", k=K)
                    nc.vector.tensor_reduce(
                        out=pf[:, mo2:mo2 + m_run, OUT_C:OUT_C + 1],
                        in_=wv, axis=AX.X, op=OP.add)
                    co2 += m_run * K
                    mo2 += m_run
                partial = ph2b.tile([128, MAXM, EL], bf16, tag="partial")
                nc.vector.tensor_copy(partial[:, 0:mb, 0:OUT_C + 1],
                                      pf[:, 0:mb, :])
                nc.gpsimd.dma_scatter_add(
                    out_ap=(acc0 if k % 2 == 0 else acc1)[:],
                    in_ap=partial[:, 0:mb, :],
                    idxs_ap=sit[:, 0:mcols16],
                    num_idxs=128 * mb, num_idxs_reg=128 * mb,
                    elem_size=EL, single_packet=False)

            SKEW = 2
            hg_tiles = {}
            for k in range(nbat + SKEW):
                if k < nbat:
                    hg_tiles[k] = emit_gather(k)
                if k >= SKEW:
                    emit_compute_scatter(k - SKEW, hg_tiles.pop(k - SKEW))

            # ---------- final: out = (h'0+h'1) / (e0+e1) (chunked) ----------
            nout_cols = NPC_PAD // 128   # 98
            chunk = 25
            for c0 in range(0, nout_cols, chunk):
                cn = min(chunk, nout_cols - c0)
                at0 = fin.tile([128, chunk, EL], bf16, tag="at0")
                at1 = fin.tile([128, chunk, EL], bf16, tag="at1")
                for at, accd in ((at0, acc0), (at1, acc1)):
                    nc.sync.dma_start(
                        at[:, 0:cn, :],
                        accd[c0 * 128:(c0 + cn) * 128, :].rearrange(
                            "(j p) e -> p j e", p=128))
                esum = fin.tile([128, chunk], f32, tag="esum")
                nc.vector.tensor_tensor(
                    out=esum[:, 0:cn, None],
                    in0=at0[:, 0:cn, OUT_C:OUT_C + 1],
                    in1=at1[:, 0:cn, OUT_C:OUT_C + 1], op=OP.add)
                rec = fin.tile([128, chunk], f32, tag="rec")
                nc.vector.reciprocal(rec[:, 0:cn], esum[:, 0:cn])
                hsum = fin.tile([128, chunk, OUT_C], f32, tag="hsum")
                nc.vector.tensor_tensor(
                    out=hsum[:, 0:cn, :], in0=at0[:, 0:cn, 0:OUT_C],
                    in1=at1[:, 0:cn, 0:OUT_C], op=OP.add)
                ot = fin.tile([128, chunk, OUT_C], f32, tag="ot")
                nc.vector.tensor_tensor(
                    out=ot[:, 0:cn, :], in0=hsum[:, 0:cn, :],
                    in1=rec[:, 0:cn, None].to_broadcast([128, cn, OUT_C]),
                    op=OP.mult)
                nc.sync.dma_start(
                    out[c0 * 128:(c0 + cn) * 128, :].rearrange(
                        "(j p) e -> p j e", p=128), ot[:, 0:cn, :])
    nc.compile()
    return nc


_CACHE = {}
_LAST = {}  # debug/timing introspection: nc + in_maps of last call


def kernel(x, W, a, edge_index):
    from concourse.bass_utils import run_bass_kernel_spmd

    x = np.asarray(x, np.float32)
    W = np.asarray(W, np.float32)
    a = np.asarray(a, np.float32)
    meta, per_core = _prep(edge_index)

    key = (N, E, tuple((b["sec"], tuple(b["runs"])) for b in meta["batches"]))
    if key not in _CACHE:
        _CACHE[key] = _build_program(meta)
    nc = _CACHE[key]

    xTf = np.ascontiguousarray(x.T)  # [128, N]
    aTv = np.ascontiguousarray(a.reshape(2, OUT_C).T)  # [64, 2]
    mcols_total = meta["mcols"]
    mcols_pad = -(-mcols_total // GB) * GB
    in_maps = []
    for c in range(NCORES):
        n0 = c * NPC
        xrot = np.concatenate(
            [xTf[:, n0:], xTf[:, :n0],
             np.zeros((128, XP - N), np.float32)], axis=1)
        ids = per_core[c]["sorder"].T.reshape(-1)  # q = j*128+p
        xs = np.zeros((mcols_pad * 128, IN_C), np.float32)
        valid = ids < NPC
        xs[:len(ids)][valid] = x[n0 + ids[valid]]
        in_maps.append({
            "xT": xrot, "W": W, "aT": aTv,
            "xS": np.ascontiguousarray(xs.T),
            "gidx": per_core[c]["gidx"],
            "sidx": per_core[c]["sidx"],
        })
    _LAST["nc"] = nc
    _LAST["in_maps"] = in_maps
    res = run_bass_kernel_spmd(nc, in_maps, core_ids=list(range(NCORES)))
    outs = [res.results[c]["out"][:NPC] for c in range(NCORES)]
    return np.concatenate(outs, axis=0)



# revision 26
# speedup vs baseline: 29.7649x; 1.1636x over previous
"""GAT layer (message passing) on 8 Trainium2 NeuronCores via Bass/Tile.

Strategy (src-sharded, dst-sectioned, K-padded node-aligned slots):
  - 8 cores, each owns nodes [c*NPC, (c+1)*NPC) (src sharding -> segment
    sums stay core-local).
  - Host passes x^T ROTATED per core so the core's own shard maps to table
    rows [0, NPC): all program constants become core-independent; per-core
    differences live only in index input data.
  - Phase 1 (per core, replicated work): compute per-node row
    G[n] = [h(64xbf16) | t(f32) | pad] via TensorE from xT chunks,
    where h = x@W, t = h@a2.  G lives in DRAM as one tile PER SECTION
    (<=25088 rows + a TRASH row) so int16 dma_gather indices stay in
    range and phase-2 gathers of section s only wait on section s's
    writes.  Phase 1b (s = h@a1 in slot-node order, from host-gathered
    xS) is interleaved per section right after that section's phase 1,
    writing per-section s tiles, so early batches unblock asap.
  - Phase 2, software-pipelined with a 2-batch emission skew: per batch,
    dma_gather slot rows (256B) by dst (SWDGE queue 0), compute
    w = exp(leaky_relu(s+t)) and prod = w*h on ACT/DVE, reduce each
    node's K slots to f32 (DVE), cast to a bf16 row [h'(64)|e_sum|pad]
    and dma_scatter_add (256B rows, SWDGE queue 1) into one of TWO bf16
    DRAM accumulators (even/odd batches).  Within one accumulator,
    scatters are serialized by partial-buffer reuse; across the two,
    rows never collide -> no RMW races, transfers overlap.
    Pool-engine SWDGE descriptor generation (~7.3ns/descriptor) is the
    kernel's critical path; everything else overlaps under it.
  - Final: out = (h'0 + h'1) / (e0 + e1) in f32, chunked.
"""

import numpy as np

N = 100000
E = 1600000
IN_C = 128
OUT_C = 64
ALPHA = 0.2
NCORES = 8
EL = 128                       # bf16 elements per table row (256B)
GB = 4                         # phase-1 chunks (of 128 nodes) per group
MAXC = 64                      # max slot-columns per batch (nidx<=8192)
MAXM = 64                      # max node-columns per batch
K_BUCKETS = (1, 2, 3, 4, 5, 6, 8, 10, 12, 14, 16, 20, 24, 28, 32,
             40, 48, 64, 96, 128)


def _configure(n=100000, e=1600000, secr_cap=25088):
    """Set the graph-size-derived constants (module globals)."""
    global N, E, NPC, NPC_PAD, ACC_TRASH, ACC_ROWS, SECR, NSEC
    global SECSTRIDE, XP, TROWS, XTILE
    N, E = n, e
    NPC = N // NCORES
    NPC_PAD = ((NPC + 127) // 128) * 128
    ACC_TRASH = NPC_PAD
    ACC_ROWS = NPC_PAD + 128
    NSEC = max(1, -(-N // secr_cap))
    per_sec = -(-N // NSEC)
    SECR = ((per_sec + 511) // 512) * 512   # multiple of 512 for phase-1 GB
    SECSTRIDE = SECR + 128
    XP = NSEC * SECR
    TROWS = NSEC * SECSTRIDE
    XTILE = min(2048, XP)
    assert XP % (128 * GB) == 0
    assert NPC_PAD < SECR + 128, "s-gather idx must stay in section 0 range"


_configure()


def _wrap16(vals):
    """[128, cols] rank layout (rank i -> (p=i%128, col=i//128)) ->
    dma_gather idx tensor [128, (128*cols)/16] int16 (wrapped, replicated)."""
    L = vals.T.reshape(-1)  # rank order
    n = L.shape[0]
    w = L.reshape(n // 16, 16).T  # [16, n/16]
    return np.tile(w, (8, 1)).astype(np.int16)


def _bucket(d):
    for k in K_BUCKETS:
        if d <= k:
            return k
    raise ValueError(f"degree {d} exceeds max bucket")


def _prep(edge_index):
    """Pure-integer host prep: per-core rotated sections, classes, batches.

    Returns (meta, per_core) where meta is core-independent (defines the
    program) and per_core holds the index input tensors."""
    src = np.asarray(edge_index[0], dtype=np.int64)
    dst = np.asarray(edge_index[1], dtype=np.int64)
    loops = np.arange(N, dtype=np.int64)
    src = np.concatenate([src, loops])
    dst = np.concatenate([dst, loops])

    core_of = src // NPC
    kb = np.asarray(K_BUCKETS)

    # ---- pass 1: per-core degree tables in rotated-section space ----
    per_core_raw = []
    # counts_psk[c][sec][ki][p] = #nodes of partition p in class ki
    counts_psk = np.zeros((NCORES, NSEC, len(kb), 128), np.int64)
    for c in range(NCORES):
        m = core_of == c
        sl = (src[m] - c * NPC).astype(np.int64)
        rot = (dst[m] - c * NPC) % N  # rotated dst position
        sec = rot // SECR
        row = (rot % SECR).astype(np.int32)
        key = sl * NSEC + sec
        order = np.argsort(key, kind="stable")
        rows_sorted = row[order]
        cnt = np.bincount(key, minlength=NPC * NSEC).reshape(NPC, NSEC)
        offs = np.zeros(NPC * NSEC + 1, np.int64)
        np.cumsum(cnt.ravel(), out=offs[1:])
        # class per (node, sec); -1 when no edges in that section
        cls = np.full((NPC, NSEC), -1, np.int64)
        nz = cnt > 0
        cls[nz] = np.searchsorted(kb, cnt[nz])
        for s_ in range(NSEC):
            for ki in range(len(kb)):
                n_in_class = int((cls[:, s_] == ki).sum())
                # nodes are distributed round-robin across partitions, so
                # per-partition count is at most ceil(n/128)
                counts_psk[c, s_, ki] = -(-n_in_class // 128)
        per_core_raw.append((cnt, cls, rows_sorted, offs))

    # ---- shared metadata: padded class sizes + batch structure ----
    m_sk = counts_psk.max(axis=(0, 3))  # [NSEC, nK]
    total_slots = int((m_sk * kb[None, :]).sum() * 128)
    total_edges = E + N
    # batches: list of dicts(sec, runs=[(ki, m_run)], cols, m)
    batches = []
    for s_ in range(NSEC):
        cur = {"sec": s_, "runs": [], "cols": 0, "m": 0}
        for ki in reversed(range(len(kb))):  # big K first
            K = int(kb[ki])
            m_left = int(m_sk[s_, ki])
            while m_left > 0:
                mfit_cols = (MAXC - cur["cols"]) // K
                mfit = min(m_left, mfit_cols, MAXM - cur["m"])
                if mfit <= 0:
                    if cur["runs"]:
                        batches.append(cur)
                    cur = {"sec": s_, "runs": [], "cols": 0, "m": 0}
                    continue
                cur["runs"].append((ki, mfit))
                cur["cols"] += mfit * K
                cur["m"] += mfit
                m_left -= mfit
        if cur["runs"]:
            batches.append(cur)
    meta = {"m_sk": m_sk, "batches": batches,
            "gcols": sum(b["cols"] for b in batches),
            "mcols": sum(b["m"] for b in batches),
            "inflation": total_slots / total_edges * NCORES}

    # ---- pass 2: fill per-core index tensors ----
    per_core = []
    for c in range(NCORES):
        cnt, cls, rows_sorted, offs = per_core_raw[c]
        p_of = np.arange(NPC) % 128
        # per (sec, ki): nodes grouped by partition, padded to m_sk
        gidx_blocks = []
        sidx_blocks = []
        nidx_blocks = []
        for b in batches:
            s_ = b["sec"]
            gmat = np.full((128, b["cols"]), SECR, np.int64)  # TRASH row
            smat = np.full((128, b["m"]), ACC_TRASH, np.int64)
            nmat = np.zeros((128, b["m"]), np.int64)  # s-gather idx
            b["_fill"] = (gmat, smat, nmat)
        run_index = []  # [(batch, col_off, m_off, ki, m_run)]
        for b in batches:
            co, mo = 0, 0
            for ki, m_run in b["runs"]:
                run_index.append((b, co, mo, ki, m_run))
                co += m_run * int(kb[ki])
                mo += m_run
        # group nodes per (sec, ki) with round-robin partition assignment:
        # node rank r in the class -> partition r%128, column r//128
        for s_ in range(NSEC):
            for ki in range(len(kb)):
                sel = np.where(cls[:, s_] == ki)[0]
                ncl = -(-len(sel) // 128) if len(sel) else 0
                nodes_by_p = [sel[p::128] for p in range(128)]
                K = int(kb[ki])
                consumed = np.zeros(128, np.int64)
                for b, co, mo, ki2, m_run in run_index:
                    if b["sec"] != s_ or ki2 != ki:
                        continue
                    gmat, smat, nmat = b["_fill"]
                    for p in range(128):
                        av = nodes_by_p[p][consumed[p]:consumed[p] + m_run]
                        nn = len(av)
                        if nn == 0:
                            continue
                        # node columns mo..mo+nn-1 on partition p
                        smat[p, mo:mo + nn] = av
                        nmat[p, mo:mo + nn] = av
                        # gather slots
                        d = cnt[av, s_]
                        base = offs[av * NSEC + s_]
                        idxmat = base[:, None] + np.arange(K)[None, :]
                        valid = np.arange(K)[None, :] < d[:, None]
                        vals = np.where(
                            valid,
                            rows_sorted[np.minimum(idxmat,
                                                   len(rows_sorted) - 1)],
                            SECR)
                        gmat[p, co + 0:co + nn * K] = vals.reshape(-1)
                    consumed += m_run
        # wrap all batches
        sorder_blocks = []
        for b in batches:
            gmat, smat, nmat = b["_fill"]
            gidx_blocks.append(_wrap16(gmat))
            sidx_blocks.append(_wrap16(smat))
            sorder_blocks.append(smat)
        per_core.append({
            "gidx": np.concatenate(gidx_blocks, axis=1),
            "sidx": np.concatenate(sidx_blocks, axis=1),
            "sorder": np.concatenate(sorder_blocks, axis=1),
        })
        for b in batches:
            del b["_fill"]
    return meta, per_core


def _build_program(meta, repeat=1, rep_mode="both", ablate=()):
    import concourse.bacc as bacc
    import concourse.tile as tile
    from concourse import mybir
    from concourse.masks import make_identity

    f32 = mybir.dt.float32
    bf16 = mybir.dt.bfloat16
    i16 = mybir.dt.int16
    AF = mybir.ActivationFunctionType
    OP = mybir.AluOpType
    AX = mybir.AxisListType
    kb = K_BUCKETS

    batches = meta["batches"]
    gcols_total = meta["gcols"]
    mcols_total = meta["mcols"]
    nbat = len(batches)
    # prefix offsets per batch (wrapped-idx cols and node cols)
    goffs = np.concatenate([[0], np.cumsum([b["cols"] * 8 for b in batches])])
    moffs = np.concatenate([[0], np.cumsum([b["m"] * 8 for b in batches])])

    nc = bacc.Bacc(None, target_bir_lowering=False, num_swdge_queues=2)
    xT = nc.dram_tensor("xT", [128, XP], f32, kind="ExternalInput")
    Wd = nc.dram_tensor("W", [IN_C, OUT_C], f32, kind="ExternalInput")
    aT = nc.dram_tensor("aT", [OUT_C, 2], f32, kind="ExternalInput")
    gidx = nc.dram_tensor("gidx", [128, gcols_total * 8], i16,
                          kind="ExternalInput")
    sidx = nc.dram_tensor("sidx", [128, mcols_total * 8], i16,
                          kind="ExternalInput")
    mcols_pad = -(-mcols_total // GB) * GB
    xS = nc.dram_tensor("xS", [128, mcols_pad * 128], f32,
                        kind="ExternalInput")
    # bf16 accumulator rows: [h'(64) | e_sum | pad] = 128 bf16 = 256B.
    # Two accumulators (even/odd batches): concurrent in-flight scatter_adds
    # never touch the same tensor, and same-parity scatters are serialized
    # by partial-buffer reuse (bufs=2) -> no RMW races.
    acc0 = nc.dram_tensor("acc0", [ACC_ROWS, EL], bf16, kind="ExternalOutput")
    acc1 = nc.dram_tensor("acc1", [ACC_ROWS, EL], bf16, kind="ExternalOutput")
    out = nc.dram_tensor("out", [NPC_PAD, OUT_C], f32, kind="ExternalOutput")

    with tile.TileContext(nc) as tc:
        with (
            tc.tile_pool(name="dram", bufs=1, space="DRAM") as dpool,
            tc.tile_pool(name="setup", bufs=1) as setup,
            tc.tile_pool(name="xin", bufs=2) as xin,
            tc.tile_pool(name="ps", bufs=2, space="PSUM") as psp,
            tc.tile_pool(name="gout", bufs=2) as gop,
            tc.tile_pool(name="ph2", bufs=3) as ph2,
            tc.tile_pool(name="hgp", bufs=4) as hgp,
            tc.tile_pool(name="ph2b", bufs=2) as ph2b,
            tc.tile_pool(name="pfp", bufs=1) as pfp,
            tc.tile_pool(name="fin", bufs=1) as fin,
        ):
            Gs = []
            for s_ in range(NSEC):
                gsec = dpool.tile([SECSTRIDE, EL], bf16, tag=f"gsec{s_}",
                                  name=f"gsec{s_}")
                Gs.append(gsec)

            # ---------- setup: W, Wa1, Wa2 ----------
            ident = setup.tile([128, 128], f32)
            make_identity(nc, ident[:])
            Wt = setup.tile([128, OUT_C], f32)
            nc.sync.dma_start(Wt[:], Wd[:])
            aTt = setup.tile([OUT_C, 2], f32)
            nc.sync.dma_start(aTt[:], aT[:])
            WT_ps = psp.tile([OUT_C, 128], f32, tag="wt")
            nc.tensor.transpose(out=WT_ps[:], in_=Wt[:], identity=ident[:])
            WT = setup.tile([OUT_C, 128], f32)
            nc.vector.tensor_copy(WT[:], WT_ps[:])
            Wa_ps = psp.tile([128, 2], f32, tag="wa")
            nc.tensor.matmul(Wa_ps[:], WT[:], aTt[:], start=True, stop=True)
            # rhs = [W | Wa2] (t rides in G); Wa1 separate for phase 1b (s)
            rhs = setup.tile([128, OUT_C + 1], f32)
            nc.vector.tensor_copy(rhs[:, 0:OUT_C], Wt[:])
            nc.vector.tensor_copy(rhs[:, OUT_C:OUT_C + 1], Wa_ps[:, 1:2])
            wa1 = setup.tile([128, 1], f32)
            nc.vector.tensor_copy(wa1[:], Wa_ps[:, 0:1])
            # per-section s tiles so early batches don't wait on all of 1b
            sec_start, secs = {}, []
            for k, b in enumerate(batches):
                if b["sec"] not in sec_start:
                    sec_start[b["sec"]] = moffs[k] // 8
                    secs.append(b["sec"])
            sec_end = {
                s_: (sec_start[secs[i + 1]] if i + 1 < len(secs)
                     else mcols_total)
                for i, s_ in enumerate(secs)
            }
            s_tiles = {}
            for s_ in secs:
                stile = setup.tile(
                    [128, sec_end[s_] - sec_start[s_]], f32,
                    tag=f"sres{s_}", name=f"sres{s_}")
                s_tiles[s_] = stile

            # special TRASH rows: h=0, t=-1e30, s=0
            sp = setup.tile([1, EL], bf16)
            nc.vector.memset(sp[:], 0)
            spf = sp[:].bitcast(f32)  # [1, 64]
            nc.vector.memset(spf[0:1, 32:33], -1.0e30)
            for s_ in range(NSEC):
                nc.sync.dma_start(Gs[s_][SECR:SECR + 1, :], sp[:])

            # ---------- phase 1 + 1b, section-interleaved ----------
            # p1 section s, then the 1b groups covering section s's
            # s-columns: early phase-2 batches unblock asap.
            n_sgroups = mcols_pad // GB
            groups_per_xtile = max(1, XTILE // (128 * GB))

            def emit_p1_section(s_):
                glo = s_ * (SECR // 512)
                ghi = (s_ + 1) * (SECR // 512)
                xt_t = None
                for g in range(glo, ghi):
                    if (g - glo) % groups_per_xtile == 0:
                        xt_t = xin.tile([128, XTILE], f32, tag="xt")
                        x0 = g * 128 * GB
                        xlen = min(XTILE, XP - x0)
                        nc.sync.dma_start(xt_t[:, 0:xlen],
                                          xT[:, x0:x0 + xlen])
                    ps = psp.tile([128, GB, OUT_C + 1], f32, tag="hps")
                    for j in range(GB):
                        off = ((g - glo) % groups_per_xtile) * 128 * GB + \
                            j * 128
                        nc.tensor.matmul(ps[:, j, :], xt_t[:, off:off + 128],
                                         rhs[:], start=True, stop=True)
                    gb_t = gop.tile([128, GB, EL], bf16, tag="gb")
                    nc.vector.tensor_copy(gb_t[:, :, 0:OUT_C],
                                          ps[:, :, 0:OUT_C])
                    gf = gb_t[:].bitcast(f32)  # [128, GB, 64]
                    # t (= h@a2) at f32 col 32
                    nc.vector.tensor_copy(gf[:, :, 32:33],
                                          ps[:, :, OUT_C:OUT_C + 1])
                    rowbase = (g * 512) % SECR
                    nc.sync.dma_start(
                        Gs[s_][rowbase:rowbase + 512, :].rearrange(
                            "(j p) e -> p j e", p=128),
                        gb_t[:])

            _1b_done = set()

            def emit_1b_groups(glo, ghi):
                for g in range(glo, ghi):
                    if g in _1b_done or g >= n_sgroups:
                        continue
                    _1b_done.add(g)
                    xs_t = xin.tile([128, GB * 128], f32, tag="xs")
                    nc.sync.dma_start(
                        xs_t[:], xS[:, g * GB * 128:(g + 1) * GB * 128])
                    ps2 = psp.tile([128, GB, 1], f32, tag="sps")
                    for j in range(GB):
                        nc.tensor.matmul(
                            ps2[:, j, :], xs_t[:, j * 128:(j + 1) * 128],
                            wa1[:], start=True, stop=True)
                    g0 = g * GB
                    for s_ in secs:
                        lo = max(g0, sec_start[s_])
                        hi = min(g0 + GB, sec_end[s_])
                        if lo < hi:
                            nc.vector.tensor_copy(
                                s_tiles[s_][:, lo - sec_start[s_]:
                                            hi - sec_start[s_]],
                                ps2[:, lo - g0:hi - g0, 0])

            for s_ in range(NSEC):
                emit_p1_section(s_)
                if s_ in sec_start:
                    emit_1b_groups(sec_start[s_] // GB,
                                   -(-sec_end[s_] // GB))
            emit_1b_groups(0, n_sgroups)  # any stragglers

            # ---------- phase 2 (software-pipelined, skewed emission) ------
            def emit_gather(k):
                b = batches[k]
                cols, sec = b["cols"], b["sec"]
                ncols16 = cols * 8
                git = ph2.tile([128, 512], i16, tag="git")
                nc.sync.dma_start(git[:, 0:ncols16],
                                  gidx[:, goffs[k]:goffs[k] + ncols16])
                hg = hgp.tile([128, MAXC, EL], bf16, tag="hg")
                nc.gpsimd.dma_gather(
                    out_ap=hg[:, 0:cols, :],
                    in_ap=Gs[sec][:],
                    idxs_ap=git[:, 0:cols * 8],
                    num_idxs=128 * cols, num_idxs_reg=128 * cols,
                    elem_size=EL, single_packet=False)
                return hg

            def emit_compute_scatter(k, hg):
                b = batches[k]
                cols, mb = b["cols"], b["m"]
                mcols16 = mb * 8
                mo8 = moffs[k] // 8
                sit = ph2.tile([128, 512], i16, tag="sit")
                nc.sync.dma_start(sit[:, 0:mcols16],
                                  sidx[:, moffs[k]:moffs[k] + mcols16])
                s_sec = s_tiles[b["sec"]]
                ml = mo8 - sec_start[b["sec"]]
                hgf = hg[:].bitcast(f32)     # [128, MAXC, 64]
                z = ph2.tile([128, MAXC], f32, tag="z")
                co2, mo2 = 0, 0
                for ki, m_run in b["runs"]:
                    K = int(kb[ki])
                    t4 = hgf[:, co2:co2 + m_run * K, 32:33].rearrange(
                        "p (m k) e -> p m k e", k=K)
                    s4 = s_sec[:, ml + mo2:ml + mo2 + m_run][
                        :, :, None, None]
                    z4 = z[:, co2:co2 + m_run * K].rearrange(
                        "p (m k) -> p m k", k=K)[:, :, :, None]
                    nc.vector.tensor_tensor(
                        out=z4, in0=t4,
                        in1=s4.to_broadcast([128, m_run, K, 1]), op=OP.add)
                    co2 += m_run * K
                    mo2 += m_run
                zm = ph2.tile([128, MAXC], f32, tag="zm")
                zl = ph2.tile([128, MAXC], f32, tag="zl")
                w = ph2.tile([128, MAXC], bf16, tag="w")
                prod = ph2.tile([128, MAXC, OUT_C], bf16, tag="prod")
                nc.vector.tensor_scalar_mul(zm[:, 0:cols], z[:, 0:cols],
                                            ALPHA)
                nc.vector.tensor_tensor(out=zl[:, 0:cols], in0=z[:, 0:cols],
                                        in1=zm[:, 0:cols], op=OP.max)
                nc.scalar.activation(w[:, 0:cols], zl[:, 0:cols], AF.Exp)
                nc.vector.tensor_tensor(
                    out=prod[:, 0:cols, :], in0=hg[:, 0:cols, 0:OUT_C],
                    in1=w[:, 0:cols, None].to_broadcast([128, cols, OUT_C]),
                    op=OP.mult)
                pf = pfp.tile([128, MAXM, OUT_C + 1], f32, tag="pf")
                co2, mo2 = 0, 0
                for ki, m_run in b["runs"]:
                    K = int(kb[ki])
                    pv = prod[:, co2:co2 + m_run * K, :].rearrange(
                        "p (m k) e -> p m e k", k=K)
                    nc.vector.tensor_reduce(
                        out=pf[:, mo2:mo2 + m_run, 0:OUT_C], in_=pv,
                        axis=AX.X, op=OP.add)
                    wv = w[:, co2:co2 + m_run * K].rearrange(
                        "p (m k) -> p m k", k=K)
                    nc.vector.tensor_reduce(
                        out=pf[:, mo2:mo2 + m_run, OUT_C:OUT_C + 1],
                        in_=wv, axis=AX.X, op=OP.add)
                    co2 += m_run * K
                    mo2 += m_run
                partial = ph2b.tile([128, MAXM, EL], bf16, tag="partial")
                nc.vector.tensor_copy(partial[:, 0:mb, 0:OUT_C + 1],
                                      pf[:, 0:mb, :])
                nc.gpsimd.dma_scatter_add(
                    out_ap=(acc0 if k % 2 == 0 else acc1)[:],
                    in_ap=partial[:, 0:mb, :],
                    idxs_ap=sit[:, 0:mcols16],
                    num_idxs=128 * mb, num_idxs_reg=128 * mb,
                    elem_size=EL, single_packet=False, queue_num=1)

            SKEW = 2
            hg_tiles = {}
            for k in range(nbat + SKEW):
                if k < nbat:
                    hg_tiles[k] = emit_gather(k)
                if k >= SKEW:
                    emit_compute_scatter(k - SKEW, hg_tiles.pop(k - SKEW))

            # ---------- final: out = (h'0+h'1) / (e0+e1) (chunked) ----------
            nout_cols = NPC_PAD // 128   # 98
            chunk = 25
            for c0 in range(0, nout_cols, chunk):
                cn = min(chunk, nout_cols - c0)
                at0 = fin.tile([128, chunk, EL], bf16, tag="at0")
                at1 = fin.tile([128, chunk, EL], bf16, tag="at1")
                for at, accd in ((at0, acc0), (at1, acc1)):
                    nc.sync.dma_start(
                        at[:, 0:cn, :],
                        accd[c0 * 128:(c0 + cn) * 128, :].rearrange(
                            "(j p) e -> p j e", p=128))
                esum = fin.tile([128, chunk], f32, tag="esum")
                nc.vector.tensor_tensor(
                    out=esum[:, 0:cn, None],
                    in0=at0[:, 0:cn, OUT_C:OUT_C + 1],
                    in1=at1[:, 0:cn, OUT_C:OUT_C + 1], op=OP.add)
                rec = fin.tile([128, chunk], f32, tag="rec")
                nc.vector.reciprocal(rec[:, 0:cn], esum[:, 0:cn])
                hsum = fin.tile([128, chunk, OUT_C], f32, tag="hsum")
                nc.vector.tensor_tensor(
                    out=hsum[:, 0:cn, :], in0=at0[:, 0:cn, 0:OUT_C],
                    in1=at1[:, 0:cn, 0:OUT_C], op=OP.add)
                ot = fin.tile([128, chunk, OUT_C], f32, tag="ot")
                nc.vector.tensor_tensor(
                    out=ot[:, 0:cn, :], in0=hsum[:, 0:cn, :],
                    in1=rec[:, 0:cn, None].to_broadcast([128, cn, OUT_C]),
                    op=OP.mult)
                nc.sync.dma_start(
                    out[c0 * 128:(c0 + cn) * 128, :].rearrange(
                        "(j p) e -> p j e", p=128), ot[:, 0:cn, :])
    nc.compile()
    return nc


_CACHE = {}
_LAST = {}  # debug/timing introspection: nc + in_maps of last call


def kernel(x, W, a, edge_index):
    from concourse.bass_utils import run_bass_kernel_spmd

    x = np.asarray(x, np.float32)
    W = np.asarray(W, np.float32)
    a = np.asarray(a, np.float32)
    meta, per_core = _prep(edge_index)

    key = (N, E, tuple((b["sec"], tuple(b["runs"])) for b in meta["batches"]))
    if key not in _CACHE:
        _CACHE[key] = _build_program(meta)
    nc = _CACHE[key]

    xTf = np.ascontiguousarray(x.T)  # [128, N]
    aTv = np.ascontiguousarray(a.reshape(2, OUT_C).T)  # [64, 2]
    mcols_total = meta["mcols"]
    mcols_pad = -(-mcols_total // GB) * GB
    in_maps = []
    for c in range(NCORES):
        n0 = c * NPC
        xrot = np.concatenate(
            [xTf[:, n0:], xTf[:, :n0],
             np.zeros((128, XP - N), np.float32)], axis=1)
        ids = per_core[c]["sorder"].T.reshape(-1)  # q = j*128+p
        xs = np.zeros((mcols_pad * 128, IN_C), np.float32)
        valid = ids < NPC
        xs[:len(ids)][valid] = x[n0 + ids[valid]]
        in_maps.append({
            "xT": xrot, "W": W, "aT": aTv,
            "xS": np.ascontiguousarray(xs.T),
            "gidx": per_core[c]["gidx"],
            "sidx": per_core[c]["sidx"],
        })
    _LAST["nc"] = nc
    _LAST["in_maps"] = in_maps
    res = run_bass_kernel_spmd(nc, in_maps, core_ids=list(range(NCORES)))
    outs = [res.results[c]["out"][:NPC] for c in range(NCORES)]
    return np.concatenate(outs, axis=0)

